# revision 20
# baseline (speedup 1.0000x reference)
"""GRU cell (single timestep) on 8 TRN2 NeuronCores, data-parallel over batch.

Contract: kernel(**inputs) takes FULL numpy inputs (as produced by the
problem's setup_inputs()) and returns the FULL (16384, 1024) float32 output.

Strategy:
  - Shard batch (16384) across 8 cores -> 2048 rows/core. Replicate weights.
  - Host-side packing puts every tensor in feature-major ("transposed world")
    layout so the TensorEngine contraction dim is the partition dim and no
    on-chip transposes are needed.
  - Matmul precision mix (chosen from a host-side error study against the
    f32 reference; gate is rel_fro < 2e-2):
      * r/z gates (x-side and h-side) and the candidate gate's (r*h)@Whh in
        fp8e4 with DoubleRow perf mode (2 contraction rows per PE cell,
        2x matmul throughput).
      * candidate gate's x@Wxh in bf16 (its error feeds tanh with slope ~1
        straight into the output, so it dominates the budget in fp8).
  - Scaling: fp8 operands are pre-scaled by 32 (activations) and 4096
    (weights) to sit inside fp8e4's +-240 range; every matmul contribution
    thus carries a 2^17 factor (the bf16 Wxh is pre-scaled by 2^17 to match)
    which the activation instruction removes via its input scale.
  - Perf shaping: DMAs are issued in first-use order (block-0 inputs and
    R-phase weights first) since they serialize on one HWDGE queue; a burst
    of dummy matmuls at t=0 trips the PE HAM clock-gate to 2.4 GHz before
    real data lands; HC groups alternate bf16-first/DR-first to halve
    perf-mode transitions; the whole elementwise tail runs in bf16 (2x DVE)
    and the output is returned as bf16 and upconverted on the host.
"""

import sys

if "/opt/trn_rl_repo" not in sys.path:
    sys.path.insert(0, "/opt/trn_rl_repo")

import numpy as np
import ml_dtypes

import concourse.bass as bass
import concourse.tile as tile
from concourse import bacc, mybir
from concourse.bass_utils import run_bass_kernel_spmd

P = 128
NCORES = 8
BATCH = 16384
NB = BATCH // NCORES          # 2048 rows per core
IN = 512
HID = 1024
KX = IN // P                  # 4
KH = HID // P                 # 8
M = HID // P                  # 8 output-feature chunks
BLK = 512                     # batch columns per block
NBLK = NB // BLK              # 4

SA = 32.0                     # fp8 activation scale
SW = 4096.0                   # fp8 weight scale
DEQ = 1.0 / (SA * SW)         # 2^-17, folded into the activation input scale

F32 = mybir.dt.float32
BF16 = mybir.dt.bfloat16
FP8 = mybir.dt.float8e4
DR = mybir.MatmulPerfMode.DoubleRow

E4NP = ml_dtypes.float8_e4m3  # TRN fp8e4 semantics (max +-240)
BFNP = ml_dtypes.bfloat16

_CACHE = {}


def _build():
    nc = bacc.Bacc("TRN2", target_bir_lowering=False, debug=False, num_devices=NCORES)

    xq = nc.dram_tensor("xq", [P, KX, NB], FP8, kind="ExternalInput").ap()
    xb = nc.dram_tensor("xb", [P, KX, NB], BF16, kind="ExternalInput").ap()
    hq = nc.dram_tensor("hq", [P, KH, NB], FP8, kind="ExternalInput").ap()
    h32 = nc.dram_tensor("h32", [P, KH, NB], BF16, kind="ExternalInput").ap()
    wxr = nc.dram_tensor("wxr", [P, KX, HID], FP8, kind="ExternalInput").ap()
    wxz = nc.dram_tensor("wxz", [P, KX, HID], FP8, kind="ExternalInput").ap()
    wxh = nc.dram_tensor("wxh", [P, KX, HID], BF16, kind="ExternalInput").ap()
    whr = nc.dram_tensor("whr", [P, KH, HID], FP8, kind="ExternalInput").ap()
    whz = nc.dram_tensor("whz", [P, KH, HID], FP8, kind="ExternalInput").ap()
    whh = nc.dram_tensor("whh", [P, KH, HID], FP8, kind="ExternalInput").ap()
    bias = nc.dram_tensor("bias", [P, 24], F32, kind="ExternalInput").ap()
    outT = nc.dram_tensor("outT", [P, M, NB], BF16, kind="ExternalOutput").ap()

    with tile.TileContext(nc) as tc:
        with (
            tc.tile_pool(name="warm", bufs=1) as warmpool,
            tc.tile_pool(name="wpool", bufs=1) as wpool,
            tc.tile_pool(name="xpool", bufs=2) as xpool,
            tc.tile_pool(name="hpool", bufs=2) as hpool,
            tc.tile_pool(name="rpool", bufs=2) as rpool,
            tc.tile_pool(name="rhpool", bufs=2) as rhpool,
            tc.tile_pool(name="zpool", bufs=2) as zpool,
            tc.tile_pool(name="hcpool", bufs=2) as hcpool,
            tc.tile_pool(name="opool", bufs=3) as opool,
            tc.tile_pool(name="psum", bufs=8, space=bass.MemorySpace.PSUM) as psum,
        ):
            # ---- PE warmup: ~4us of dummy DoubleRow matmuls with no DMA deps.
            # Keeps the PE busy through the DMA head so the HAM clock-gate is
            # at 8/8 (2.4 GHz) when the real stream starts.
            wu_w = warmpool.tile([P, 2, P], FP8)
            wu_x = warmpool.tile([P, 2, BLK], FP8)
            nc.gpsimd.memset(wu_w[:], 0.0)
            nc.gpsimd.memset(wu_x[:], 0.0)
            wu_ps = psum.tile([P, BLK], F32, tag="ps")
            for _ in range(9):
                nc.tensor.matmul(
                    wu_ps[:], wu_w[:], wu_x[:], start=True, stop=True, perf_mode=DR
                )

            # resident weights + bias (tiles only; DMAs are interleaved with
            # the block-0 input loads below in first-use order). The R-phase
            # weights are split into lo/hi output halves so the first matmul
            # only waits on half the weight bytes.
            wxr_lo = wpool.tile([P, KX, HID // 2], FP8)
            wxr_hi = wpool.tile([P, KX, HID // 2], FP8)
            whr_lo = wpool.tile([P, KH, HID // 2], FP8)
            whr_hi = wpool.tile([P, KH, HID // 2], FP8)
            wxz_s = wpool.tile([P, KX, HID], FP8)
            wxh_s = wpool.tile([P, KX, HID], BF16)
            whz_s = wpool.tile([P, KH, HID], FP8)
            whh_s = wpool.tile([P, KH, HID], FP8)
            b_s = wpool.tile([P, 24], F32)

            for blk in range(NBLK):
                sl = bass.ts(blk, BLK)
                xq_b = xpool.tile([P, KX, BLK], FP8, tag="xq")
                nc.sync.dma_start(xq_b[:], xq[:, :, sl])
                hq_b = hpool.tile([P, KH, BLK], FP8, tag="hq")
                nc.sync.dma_start(hq_b[:], hq[:, :, sl])
                if blk == 0:
                    nc.sync.dma_start(b_s[:], bias[:])
                    nc.sync.dma_start(wxr_lo[:], wxr[:, :, : HID // 2])
                    nc.sync.dma_start(whr_lo[:], whr[:, :, : HID // 2])
                    nc.sync.dma_start(wxr_hi[:], wxr[:, :, HID // 2 :])
                    nc.sync.dma_start(whr_hi[:], whr[:, :, HID // 2 :])
                h32_b = hpool.tile([P, KH, BLK], BF16, tag="h32")
                nc.sync.dma_start(h32_b[:], h32[:, :, sl])
                if blk == 0:
                    nc.sync.dma_start(wxz_s[:], wxz[:])
                    nc.sync.dma_start(whz_s[:], whz[:])
                xb_b = xpool.tile([P, KX, BLK], BF16, tag="xb")
                nc.sync.dma_start(xb_b[:], xb[:, :, sl])
                if blk == 0:
                    nc.sync.dma_start(wxh_s[:], wxh[:])
                    nc.sync.dma_start(whh_s[:], whh[:])

                rh = rhpool.tile([P, KH, BLK], FP8, tag="rh")
                zf = zpool.tile([P, M, BLK], BF16, tag="zf")
                zh = zpool.tile([P, M, BLK], BF16, tag="zh")
                zc = zpool.tile([P, M, BLK], BF16, tag="zc")

                # ---- R phase: r = sigmoid((x@Wxr + h@Whr)*DEQ + bxr); rh = fp8(r*32h)
                for m in range(M):
                    ps = psum.tile([P, BLK], F32, tag="ps")
                    wxr_h = wxr_lo if m < M // 2 else wxr_hi
                    whr_h = whr_lo if m < M // 2 else whr_hi
                    mo = bass.ts(m % (M // 2), P)
                    for k in range(KX // 2):
                        nc.tensor.matmul(
                            ps[:], wxr_h[:, 2 * k : 2 * k + 2, mo],
                            xq_b[:, 2 * k : 2 * k + 2, :],
                            start=(k == 0), stop=False, perf_mode=DR,
                        )
                    for k in range(KH // 2):
                        nc.tensor.matmul(
                            ps[:], whr_h[:, 2 * k : 2 * k + 2, mo],
                            hq_b[:, 2 * k : 2 * k + 2, :],
                            start=False, stop=(k == KH // 2 - 1), perf_mode=DR,
                        )
                    rt = rpool.tile([P, BLK], BF16, tag="rt")
                    nc.scalar.activation(
                        rt[:], ps[:], mybir.ActivationFunctionType.Sigmoid,
                        bias=b_s[:, m : m + 1], scale=DEQ,
                    )
                    nc.vector.tensor_mul(rh[:, m, :], rt[:], h32_b[:, m, :])

                # ---- Z phase: z = sigmoid(x@Wxz + bxz + h@Whz)
                for m in range(M):
                    ps = psum.tile([P, BLK], F32, tag="ps")
                    mo = bass.ts(m, P)
                    for k in range(KX // 2):
                        nc.tensor.matmul(
                            ps[:], wxz_s[:, 2 * k : 2 * k + 2, mo],
                            xq_b[:, 2 * k : 2 * k + 2, :],
                            start=(k == 0), stop=False, perf_mode=DR,
                        )
                    for k in range(KH // 2):
                        nc.tensor.matmul(
                            ps[:], whz_s[:, 2 * k : 2 * k + 2, mo],
                            hq_b[:, 2 * k : 2 * k + 2, :],
                            start=False, stop=(k == KH // 2 - 1), perf_mode=DR,
                        )
                    nc.scalar.activation(
                        zf[:, m, :], ps[:], mybir.ActivationFunctionType.Sigmoid,
                        bias=b_s[:, 8 + m : 9 + m], scale=DEQ,
                    )
                    # Precompute z*h and (1-z) here so the post-tanh DVE chain
                    # in the HC phase (the kernel's tail) is only 2 ops.
                    nc.vector.scalar_tensor_tensor(
                        zh[:, m, :], h32_b[:, m, :], 1.0 / SA, zf[:, m, :],
                        op0=mybir.AluOpType.mult, op1=mybir.AluOpType.mult,
                    )
                    nc.vector.tensor_scalar(
                        zc[:, m, :], zf[:, m, :], -1.0, 1.0,
                        op0=mybir.AluOpType.mult, op1=mybir.AluOpType.add,
                    )

                # ---- HC phase: hc = tanh(x@Wxh(bf16) + (r*h)@Whh(fp8-DR)).
                # Alternate bf16-first / DR-first per m so consecutive groups
                # keep the same PE perf mode across the group boundary.
                for m in range(M):
                    ps = psum.tile([P, BLK], F32, tag="ps")
                    mo = bass.ts(m, P)

                    seq = [("x", k) for k in range(KX)] + [
                        ("r", k) for k in range(KH // 2)
                    ]
                    if m % 2 == 1:
                        seq.reverse()
                    for i, (kind, k) in enumerate(seq):
                        st, sp = (i == 0), (i == len(seq) - 1)
                        if kind == "x":
                            nc.tensor.matmul(
                                ps[:], wxh_s[:, k, mo], xb_b[:, k, :],
                                start=st, stop=sp,
                            )
                        else:
                            nc.tensor.matmul(
                                ps[:], whh_s[:, 2 * k : 2 * k + 2, mo],
                                rh[:, 2 * k : 2 * k + 2, :],
                                start=st, stop=sp, perf_mode=DR,
                            )

                    hct = hcpool.tile([P, BLK], BF16, tag="hct")
                    nc.scalar.activation(
                        hct[:], ps[:], mybir.ActivationFunctionType.Tanh,
                        bias=b_s[:, 16 + m : 17 + m], scale=DEQ,
                    )
                    ot = opool.tile([P, BLK], BF16, tag="ot")
                    # ot = hc*(1-z) + z*h, with z*h and (1-z) precomputed
                    nc.vector.tensor_mul(ot[:], hct[:], zc[:, m, :])
                    nc.vector.tensor_add(ot[:], ot[:], zh[:, m, :])
                    nc.sync.dma_start(outT[:, m, sl], ot[:])

    nc.compile()
    return nc


def _pack_feature_major(a: np.ndarray, nchunks: int, dtype) -> np.ndarray:
    # [rows, cols] -> [128, nchunks, cols] with [p, k, c] = a[128k+p, c]
    rows, cols = a.shape
    assert rows == nchunks * P
    return np.ascontiguousarray(
        a.reshape(nchunks, P, cols).transpose(1, 0, 2)
    ).astype(dtype)


def _q8(a: np.ndarray) -> np.ndarray:
    return np.clip(a, -240.0, 240.0).astype(E4NP)


def pack_inputs(inputs: dict) -> list[dict]:
    """FULL f32 inputs -> per-core input maps (host-side shard + quantize)."""
    Wxr, Wxz, Wxh = inputs["Wxr"], inputs["Wxz"], inputs["Wxh"]
    Whr, Whz, Whh = inputs["Whr"], inputs["Whz"], inputs["Whh"]
    bxr, bxz, bxh = inputs["bxr"], inputs["bxz"], inputs["bxh"]

    wxr_p = _q8(_pack_feature_major(np.asarray(Wxr, np.float32) * SW, KX, np.float32))
    wxz_p = _q8(_pack_feature_major(np.asarray(Wxz, np.float32) * SW, KX, np.float32))
    wxh_p = _pack_feature_major(np.asarray(Wxh, np.float32) * (SA * SW), KX, BFNP)
    whr_p = _q8(_pack_feature_major(np.asarray(Whr, np.float32) * SW, KH, np.float32))
    whz_p = _q8(_pack_feature_major(np.asarray(Whz, np.float32) * SW, KH, np.float32))
    whh_p = _q8(_pack_feature_major(np.asarray(Whh, np.float32) * SW, KH, np.float32))
    bias_p = np.ascontiguousarray(
        np.concatenate(
            [np.asarray(b, np.float32).reshape(M, P).T for b in (bxr, bxz, bxh)],
            axis=1,
        )
    )  # [128, 24]

    x = np.asarray(inputs["x"], np.float32)
    hidden = np.asarray(inputs["hidden"], np.float32)

    in_maps = []
    for c in range(NCORES):
        rows = slice(c * NB, (c + 1) * NB)
        xT = np.ascontiguousarray(x[rows].T)        # [512, 2048]
        hT = np.ascontiguousarray(hidden[rows].T)   # [1024, 2048]
        in_maps.append(
            {
                "xq": _q8(_pack_feature_major(xT * SA, KX, np.float32)),
                "xb": _pack_feature_major(xT, KX, BFNP),
                "hq": _q8(_pack_feature_major(hT * SA, KH, np.float32)),
                "h32": _pack_feature_major(hT * SA, KH, BFNP),
                "wxr": wxr_p,
                "wxz": wxz_p,
                "wxh": wxh_p,
                "whr": whr_p,
                "whz": whz_p,
                "whh": whh_p,
                "bias": bias_p,
            }
        )
    return in_maps


def kernel(x, hidden, Wxr, bxr, Whr, Wxz, bxz, Whz, Wxh, bxh, Whh):
    if "nc" not in _CACHE:
        _CACHE["nc"] = _build()
    nc = _CACHE["nc"]

    in_maps = pack_inputs(
        {
            "x": x, "hidden": hidden,
            "Wxr": Wxr, "bxr": bxr, "Whr": Whr,
            "Wxz": Wxz, "bxz": bxz, "Whz": Whz,
            "Wxh": Wxh, "bxh": bxh, "Whh": Whh,
        }
    )
    _CACHE["in_maps"] = in_maps

    res = run_bass_kernel_spmd(nc, in_maps, core_ids=list(range(NCORES)))

    out = np.empty((BATCH, HID), np.float32)
    for c in range(NCORES):
        oT = np.asarray(res.results[c]["outT"], dtype=np.float32)  # [128, 8, 2048]
        out[c * NB : (c + 1) * NB] = oT.transpose(1, 0, 2).reshape(HID, NB).T
    return out


# revision 24
# speedup vs baseline: 1.0052x; 1.0052x over previous
"""GRU cell (single timestep) on 8 TRN2 NeuronCores, data-parallel over batch.

Contract: kernel(**inputs) takes FULL numpy inputs (as produced by the
problem's setup_inputs()) and returns the FULL (16384, 1024) float32 output.

Strategy:
  - Shard batch (16384) across 8 cores -> 2048 rows/core. Replicate weights.
  - Host-side packing puts every tensor in feature-major ("transposed world")
    layout so the TensorEngine contraction dim is the partition dim and no
    on-chip transposes are needed.
  - Matmul precision mix (chosen from a host-side error study against the
    f32 reference; gate is rel_fro < 2e-2):
      * r/z gates (x-side and h-side) and the candidate gate's (r*h)@Whh in
        fp8e4 with DoubleRow perf mode (2 contraction rows per PE cell,
        2x matmul throughput).
      * candidate gate's x@Wxh in bf16 (its error feeds tanh with slope ~1
        straight into the output, so it dominates the budget in fp8).
  - Scaling: fp8 operands are pre-scaled by 32 (activations) and 4096
    (weights) to sit inside fp8e4's +-240 range; every matmul contribution
    thus carries a 2^17 factor (the bf16 Wxh is pre-scaled by 2^17 to match)
    which the activation instruction removes via its input scale.
  - Perf shaping: DMAs are issued in first-use order (block-0 inputs and
    R-phase weights first) since they serialize on one HWDGE queue; a burst
    of dummy matmuls at t=0 trips the PE HAM clock-gate to 2.4 GHz before
    real data lands; HC groups alternate bf16-first/DR-first to halve
    perf-mode transitions; the whole elementwise tail runs in bf16 (2x DVE)
    and the output is returned as bf16 and upconverted on the host.
"""

import sys

if "/opt/trn_rl_repo" not in sys.path:
    sys.path.insert(0, "/opt/trn_rl_repo")

import numpy as np
import ml_dtypes

import concourse.bass as bass
import concourse.tile as tile
from concourse import bacc, mybir
from concourse.bass_utils import run_bass_kernel_spmd

P = 128
NCORES = 8
BATCH = 16384
NB = BATCH // NCORES          # 2048 rows per core
IN = 512
HID = 1024
KX = IN // P                  # 4
KH = HID // P                 # 8
M = HID // P                  # 8 output-feature chunks
BLK = 512                     # batch columns per block
NBLK = NB // BLK              # 4

SA = 32.0                     # fp8 activation scale
SW = 4096.0                   # fp8 weight scale
DEQ = 1.0 / (SA * SW)         # 2^-17, folded into the activation input scale

F32 = mybir.dt.float32
BF16 = mybir.dt.bfloat16
FP8 = mybir.dt.float8e4
DR = mybir.MatmulPerfMode.DoubleRow

E4NP = ml_dtypes.float8_e4m3  # TRN fp8e4 semantics (max +-240)
BFNP = ml_dtypes.bfloat16

_CACHE = {}


def _build():
    nc = bacc.Bacc("TRN2", target_bir_lowering=False, debug=False, num_devices=NCORES)

    xq = nc.dram_tensor("xq", [P, KX, NB], FP8, kind="ExternalInput").ap()
    xb = nc.dram_tensor("xb", [P, KX, NB], BF16, kind="ExternalInput").ap()
    hq = nc.dram_tensor("hq", [P, KH, NB], FP8, kind="ExternalInput").ap()
    h32 = nc.dram_tensor("h32", [P, KH, NB], BF16, kind="ExternalInput").ap()
    wxr = nc.dram_tensor("wxr", [P, KX, HID], FP8, kind="ExternalInput").ap()
    wxz = nc.dram_tensor("wxz", [P, KX, HID], FP8, kind="ExternalInput").ap()
    wxh = nc.dram_tensor("wxh", [P, KX, HID], BF16, kind="ExternalInput").ap()
    whr = nc.dram_tensor("whr", [P, KH, HID], FP8, kind="ExternalInput").ap()
    whz = nc.dram_tensor("whz", [P, KH, HID], FP8, kind="ExternalInput").ap()
    whh = nc.dram_tensor("whh", [P, KH, HID], FP8, kind="ExternalInput").ap()
    bias = nc.dram_tensor("bias", [P, 24], F32, kind="ExternalInput").ap()
    outT = nc.dram_tensor("outT", [P, M, NB], BF16, kind="ExternalOutput").ap()

    with tile.TileContext(nc) as tc:
        with (
            tc.tile_pool(name="warm", bufs=1) as warmpool,
            tc.tile_pool(name="wpool", bufs=1) as wpool,
            tc.tile_pool(name="xpool", bufs=2) as xpool,
            tc.tile_pool(name="hpool", bufs=2) as hpool,
            tc.tile_pool(name="rpool", bufs=2) as rpool,
            tc.tile_pool(name="rhpool", bufs=2) as rhpool,
            tc.tile_pool(name="zpool", bufs=2) as zpool,
            tc.tile_pool(name="hcpool", bufs=2) as hcpool,
            tc.tile_pool(name="opool", bufs=3) as opool,
            tc.tile_pool(name="psum", bufs=8, space=bass.MemorySpace.PSUM) as psum,
        ):
            # ---- PE warmup: ~4us of dummy DoubleRow matmuls with no DMA deps.
            # Keeps the PE busy through the DMA head so the HAM clock-gate is
            # at 8/8 (2.4 GHz) when the real stream starts.
            wu_w = warmpool.tile([P, 2, P], FP8)
            wu_x = warmpool.tile([P, 2, BLK], FP8)
            nc.gpsimd.memset(wu_w[:], 0.0)
            nc.gpsimd.memset(wu_x[:], 0.0)
            wu_ps = psum.tile([P, BLK], F32, tag="ps")
            for _ in range(13):
                nc.tensor.matmul(
                    wu_ps[:], wu_w[:], wu_x[:], start=True, stop=True, perf_mode=DR
                )

            # resident weights + bias (tiles only; DMAs are interleaved with
            # the block-0 input loads below in first-use order). The R-phase
            # weights are split into lo/hi output halves so the first matmul
            # only waits on half the weight bytes.
            wxr_s = wpool.tile([P, KX, HID], FP8)
            whr_s = wpool.tile([P, KH, HID], FP8)
            wxz_s = wpool.tile([P, KX, HID], FP8)
            wxh_s = wpool.tile([P, KX, HID], BF16)
            whz_s = wpool.tile([P, KH, HID], FP8)
            whh_s = wpool.tile([P, KH, HID], FP8)
            b_s = wpool.tile([P, 24], F32)

            for blk in range(NBLK):
                sl = bass.ts(blk, BLK)
                xq_b = xpool.tile([P, KX, BLK], FP8, tag="xq")
                nc.sync.dma_start(xq_b[:], xq[:, :, sl])
                hq_b = hpool.tile([P, KH, BLK], FP8, tag="hq")
                nc.sync.dma_start(hq_b[:], hq[:, :, sl])
                if blk == 0:
                    nc.sync.dma_start(b_s[:], bias[:])
                    nc.sync.dma_start(wxr_s[:], wxr[:])
                    nc.sync.dma_start(whr_s[:], whr[:])
                h32_b = hpool.tile([P, KH, BLK], BF16, tag="h32")
                nc.sync.dma_start(h32_b[:], h32[:, :, sl])
                if blk == 0:
                    nc.sync.dma_start(wxz_s[:], wxz[:])
                    nc.sync.dma_start(whz_s[:], whz[:])
                xb_b = xpool.tile([P, KX, BLK], BF16, tag="xb")
                nc.sync.dma_start(xb_b[:], xb[:, :, sl])
                if blk == 0:
                    nc.sync.dma_start(wxh_s[:], wxh[:])
                    nc.sync.dma_start(whh_s[:], whh[:])

                rh = rhpool.tile([P, KH, BLK], FP8, tag="rh")
                zf = zpool.tile([P, M, BLK], BF16, tag="zf")
                zh = zpool.tile([P, M, BLK], BF16, tag="zh")
                zc = zpool.tile([P, M, BLK], BF16, tag="zc")

                # ---- R phase: r = sigmoid((x@Wxr + h@Whr)*DEQ + bxr); rh = fp8(r*32h)
                for m in range(M):
                    ps = psum.tile([P, BLK], F32, tag="ps")
                    mo = bass.ts(m, P)
                    for k in range(KX // 2):
                        nc.tensor.matmul(
                            ps[:], wxr_s[:, 2 * k : 2 * k + 2, mo],
                            xq_b[:, 2 * k : 2 * k + 2, :],
                            start=(k == 0), stop=False, perf_mode=DR,
                        )
                    for k in range(KH // 2):
                        nc.tensor.matmul(
                            ps[:], whr_s[:, 2 * k : 2 * k + 2, mo],
                            hq_b[:, 2 * k : 2 * k + 2, :],
                            start=False, stop=(k == KH // 2 - 1), perf_mode=DR,
                        )
                    rt = rpool.tile([P, BLK], BF16, tag="rt")
                    nc.scalar.activation(
                        rt[:], ps[:], mybir.ActivationFunctionType.Sigmoid,
                        bias=b_s[:, m : m + 1], scale=DEQ,
                    )
                    nc.vector.tensor_mul(rh[:, m, :], rt[:], h32_b[:, m, :])

                # ---- Z phase: z = sigmoid(x@Wxz + bxz + h@Whz)
                for m in range(M):
                    ps = psum.tile([P, BLK], F32, tag="ps")
                    mo = bass.ts(m, P)
                    for k in range(KX // 2):
                        nc.tensor.matmul(
                            ps[:], wxz_s[:, 2 * k : 2 * k + 2, mo],
                            xq_b[:, 2 * k : 2 * k + 2, :],
                            start=(k == 0), stop=False, perf_mode=DR,
                        )
                    for k in range(KH // 2):
                        nc.tensor.matmul(
                            ps[:], whz_s[:, 2 * k : 2 * k + 2, mo],
                            hq_b[:, 2 * k : 2 * k + 2, :],
                            start=False, stop=(k == KH // 2 - 1), perf_mode=DR,
                        )
                    nc.scalar.activation(
                        zf[:, m, :], ps[:], mybir.ActivationFunctionType.Sigmoid,
                        bias=b_s[:, 8 + m : 9 + m], scale=DEQ,
                    )
                    # Precompute z*h and (1-z) here so the post-tanh DVE chain
                    # in the HC phase (the kernel's tail) is only 2 ops.
                    nc.vector.scalar_tensor_tensor(
                        zh[:, m, :], h32_b[:, m, :], 1.0 / SA, zf[:, m, :],
                        op0=mybir.AluOpType.mult, op1=mybir.AluOpType.mult,
                    )
                    nc.vector.tensor_scalar(
                        zc[:, m, :], zf[:, m, :], -1.0, 1.0,
                        op0=mybir.AluOpType.mult, op1=mybir.AluOpType.add,
                    )

                # ---- HC phase: hc = tanh(x@Wxh(bf16) + (r*h)@Whh(fp8-DR)).
                # Alternate bf16-first / DR-first per m so consecutive groups
                # keep the same PE perf mode across the group boundary.
                for m in range(M):
                    ps = psum.tile([P, BLK], F32, tag="ps")
                    mo = bass.ts(m, P)

                    seq = [("x", k) for k in range(KX)] + [
                        ("r", k) for k in range(KH // 2)
                    ]
                    if m % 2 == 1:
                        seq.reverse()
                    for i, (kind, k) in enumerate(seq):
                        st, sp = (i == 0), (i == len(seq) - 1)
                        if kind == "x":
                            nc.tensor.matmul(
                                ps[:], wxh_s[:, k, mo], xb_b[:, k, :],
                                start=st, stop=sp,
                            )
                        else:
                            nc.tensor.matmul(
                                ps[:], whh_s[:, 2 * k : 2 * k + 2, mo],
                                rh[:, 2 * k : 2 * k + 2, :],
                                start=st, stop=sp, perf_mode=DR,
                            )

                    hct = hcpool.tile([P, BLK], BF16, tag="hct")
                    nc.scalar.activation(
                        hct[:], ps[:], mybir.ActivationFunctionType.Tanh,
                        bias=b_s[:, 16 + m : 17 + m], scale=DEQ,
                    )
                    ot = opool.tile([P, BLK], BF16, tag="ot")
                    # ot = hc*(1-z) + z*h, with z*h and (1-z) precomputed
                    nc.vector.tensor_mul(ot[:], hct[:], zc[:, m, :])
                    nc.vector.tensor_add(ot[:], ot[:], zh[:, m, :])
                    nc.sync.dma_start(outT[:, m, sl], ot[:])

    nc.compile()
    return nc


def _pack_feature_major(a: np.ndarray, nchunks: int, dtype) -> np.ndarray:
    # [rows, cols] -> [128, nchunks, cols] with [p, k, c] = a[128k+p, c]
    rows, cols = a.shape
    assert rows == nchunks * P
    return np.ascontiguousarray(
        a.reshape(nchunks, P, cols).transpose(1, 0, 2)
    ).astype(dtype)


def _q8(a: np.ndarray) -> np.ndarray:
    return np.clip(a, -240.0, 240.0).astype(E4NP)


def pack_inputs(inputs: dict) -> list[dict]:
    """FULL f32 inputs -> per-core input maps (host-side shard + quantize)."""
    Wxr, Wxz, Wxh = inputs["Wxr"], inputs["Wxz"], inputs["Wxh"]
    Whr, Whz, Whh = inputs["Whr"], inputs["Whz"], inputs["Whh"]
    bxr, bxz, bxh = inputs["bxr"], inputs["bxz"], inputs["bxh"]

    wxr_p = _q8(_pack_feature_major(np.asarray(Wxr, np.float32) * SW, KX, np.float32))
    wxz_p = _q8(_pack_feature_major(np.asarray(Wxz, np.float32) * SW, KX, np.float32))
    wxh_p = _pack_feature_major(np.asarray(Wxh, np.float32) * (SA * SW), KX, BFNP)
    whr_p = _q8(_pack_feature_major(np.asarray(Whr, np.float32) * SW, KH, np.float32))
    whz_p = _q8(_pack_feature_major(np.asarray(Whz, np.float32) * SW, KH, np.float32))
    whh_p = _q8(_pack_feature_major(np.asarray(Whh, np.float32) * SW, KH, np.float32))
    bias_p = np.ascontiguousarray(
        np.concatenate(
            [np.asarray(b, np.float32).reshape(M, P).T for b in (bxr, bxz, bxh)],
            axis=1,
        )
    )  # [128, 24]

    x = np.asarray(inputs["x"], np.float32)
    hidden = np.asarray(inputs["hidden"], np.float32)

    in_maps = []
    for c in range(NCORES):
        rows = slice(c * NB, (c + 1) * NB)
        xT = np.ascontiguousarray(x[rows].T)        # [512, 2048]
        hT = np.ascontiguousarray(hidden[rows].T)   # [1024, 2048]
        in_maps.append(
            {
                "xq": _q8(_pack_feature_major(xT * SA, KX, np.float32)),
                "xb": _pack_feature_major(xT, KX, BFNP),
                "hq": _q8(_pack_feature_major(hT * SA, KH, np.float32)),
                "h32": _pack_feature_major(hT * SA, KH, BFNP),
                "wxr": wxr_p,
                "wxz": wxz_p,
                "wxh": wxh_p,
                "whr": whr_p,
                "whz": whz_p,
                "whh": whh_p,
                "bias": bias_p,
            }
        )
    return in_maps


def kernel(x, hidden, Wxr, bxr, Whr, Wxz, bxz, Whz, Wxh, bxh, Whh):
    if "nc" not in _CACHE:
        _CACHE["nc"] = _build()
    nc = _CACHE["nc"]

    in_maps = pack_inputs(
        {
            "x": x, "hidden": hidden,
            "Wxr": Wxr, "bxr": bxr, "Whr": Whr,
            "Wxz": Wxz, "bxz": bxz, "Whz": Whz,
            "Wxh": Wxh, "bxh": bxh, "Whh": Whh,
        }
    )
    _CACHE["in_maps"] = in_maps

    res = run_bass_kernel_spmd(nc, in_maps, core_ids=list(range(NCORES)))

    out = np.empty((BATCH, HID), np.float32)
    for c in range(NCORES):
        oT = np.asarray(res.results[c]["outT"], dtype=np.float32)  # [128, 8, 2048]
        out[c * NB : (c + 1) * NB] = oT.transpose(1, 0, 2).reshape(HID, NB).T
    return out
